# revision 47
# baseline (speedup 1.0000x reference)
"""Trainium2 Bass kernel for DynamicPTTopicModeling.

Computes, per batch b (one batch per NeuronCore, 8 cores):
    qg  = relu(qz @ bw.T)            # [R=8192, G=512], contraction over d=1024
    den = max(sum_g qg, 1e-6)        # per-row L1 norm
    msg = (qg @ bw) / den            # [R, D=1024]

Sharding: batch b across the 8 NeuronCores, fully data-parallel (one batch
per core, no collectives). Host pre-swizzles qz/bw into partition-major
layouts; the output is stored bf16 and upcast on the host. Precision is
mixed against the 2e-2 tolerance: mm1's first 256 d-dims (k-chunks 0..1)
run as fp8e4 DoubleRow matmuls, the rest bf16 — measured end-to-end error
1.51e-2 (bf16-only: 3.2e-3), buying ~12us of PE stream + smaller DMAs.

The kernel is PE-streaming-bound: 1024 N=512 matmuls = ~221us at the warm
2.4 GHz floor (HW-verified 216 ns/MM cadence, zero mid-stream gaps). All
remaining tuning is startup/tail:
  - Host layouts are partition-major ([128, ...]) so every DMA has 2-16KB
    contiguous runs per partition: pair loads are one descriptor set of
    16KB runs, and the bwT ring no longer serves 1KB packets that would
    steal SDMA round-robin slots from the qzT stream at startup.
  - Pair 0 (megas 0+1) is consumed K-OUTER with all 8 PSUM banks open
    (2 qg-pool + 6 msg-pool incl. the warmup slot): each k-quarter
    (0.5MB qzT + 0.25MB bwT, ~2.1us at the ~375 GB/s HBM cap) is consumed
    the moment its completion sem fires, so the PE does real work through
    the whole DMA-bound window instead of idling on gc-outer dependencies.
  - Junk matmuls on gpsimd-memset tiles bridge t=7..12us (PE_HAM needs
    ~3.4us of activity to unthrottle 1.2->2.4 GHz) until the first quarter
    lands; 3 short N=128 ones lead so the bridge starts before the larger
    moving-operand memset completes.
  - Pair-0 relus split ACT/DVE (2+2 for mega 0) so mm2(0)'s stationary is
    ready ~1.4us earlier than 8 serial ACT relus would allow.
  - Steady state: software-pipelined one mega ahead (mm1(t+1) interleaved
    with rowsum(t)/mm2(t)); row-sums via DVE chunk-adds + 4 tiny N=1
    matmuls directly into per-partition column layout; scale applied
    during the PSUM drain copy. The final sub's store is split in halves
    with its last drain on ACT so the closing DMA is 128KB and in flight
    ~1us after the last matmul retires.
"""
from contextlib import ExitStack

import numpy as np

import concourse.bass as bass
import concourse.tile as tile
from concourse import bacc, mybir
from concourse.bass_utils import run_bass_kernel_spmd

F32 = mybir.dt.float32
BF16 = mybir.dt.bfloat16
F8E4 = mybir.dt.float8e4
AF = mybir.ActivationFunctionType
DR = mybir.MatmulPerfMode.DoubleRow
KF8 = 2              # k-chunks 0..1 (256 of 1024 d-dims) run fp8 DoubleRow
NDR23 = 12           # megas 0..11 additionally run k-chunks 2..3 as fp8 DR
                     # (error: 8 megas measured 1.837e-2, 12 megas predicted
                     # 1.982e-2 vs the 2e-2 gate — validated model, 0.4%
                     # accuracy; all 16 extrapolates to 2.12e-2 = over)

B, C, P, D, G = 8, 16, 512, 1024, 512
R = C * P            # 8192 rows per batch
MEGA = 512           # rows per mega-tile
NSUB = MEGA // 128   # 4
NMEGA = R // MEGA    # 16
NPAIR = NMEGA // 2   # 8 (qzT loads are 2 megas per tile)
KD = D // 128        # 8 d-chunks
KG = G // 128        # 4 g-chunks
EPS = 1e-6
N_CORES = 8


def build_kernel():
    nc = bacc.Bacc("TRN2", target_bir_lowering=False)
    # Host-pre-swizzled layouts: partition dim first so DMA runs are long
    # and contiguous. qzT[p, j, k, c] = qz[j*1024+c, k*128+p] (16KB/partition
    # per pair, 4KB per startup k-quarter). bwT[p, k, g] = bw[g, k*128+p]
    # (2KB per quarter).
    # d-contraction split: k-chunks 0..1 (256 dims) in fp8e4 consumed by one
    # DoubleRow matmul per (gc, mega); k-chunks 2..7 in bf16. The DR matmul
    # issues at ~408ns (HW streams the two k-halves as two passes; verified
    # identical for split and 16B-interleaved layouts), vs 432ns+overheads
    # for the two bf16 matmuls it replaces — the net ~12us win comes from
    # that plus 64 fewer issue slots and 12.5% smaller input DMAs in the
    # supply-bound startup. End-to-end error measured 1.51e-2 vs the 2e-2
    # gate (bf16-only is 3.2e-3).
    qzT_d = nc.dram_tensor(
        "qzT", [128, NPAIR, KD - KF8, 2 * MEGA], BF16, kind="ExternalInput"
    )
    # fp8 k0/k1 interleaved at 16-byte block granularity: per partition
    # [k0 c0-15][k1 c0-15][k0 c16-31]... so the DoubleRow moving AP's second
    # dim is the k-pair at stride 16 (the verifier's Num=2, Size%16==0 form
    # at the SBUF line size) instead of stride 1024 (measured 408ns/MM,
    # i.e. two full passes).
    qzT8_d = nc.dram_tensor(
        "qzT8", [128, NPAIR, 2 * MEGA // 16, KF8, 16], F8E4, kind="ExternalInput"
    )
    qzT8b_d = nc.dram_tensor(
        "qzT8b", [128, NDR23 // 2, 2 * MEGA // 16, KF8, 16], F8E4, kind="ExternalInput"
    )
    bw_d = nc.dram_tensor("bw", [G, D], BF16, kind="ExternalInput")
    bwT_d = nc.dram_tensor("bwT", [128, KD - KF8, G], BF16, kind="ExternalInput")
    bwT8_d = nc.dram_tensor("bwT8", [128, KF8, G], F8E4, kind="ExternalInput")
    bwT8b_d = nc.dram_tensor("bwT8b", [128, KF8, G], F8E4, kind="ExternalInput")
    msg_d = nc.dram_tensor("msg", [R, D], BF16, kind="ExternalOutput")

    with tile.TileContext(nc) as tc, ExitStack() as ctx:
        const_pool = ctx.enter_context(tc.tile_pool(name="const", bufs=1))
        in_pool = ctx.enter_context(tc.tile_pool(name="inp", bufs=3))
        in8_pool = ctx.enter_context(tc.tile_pool(name="inp8", bufs=3))
        in8b_pool = ctx.enter_context(tc.tile_pool(name="inp8b", bufs=3))
        # bufs=3: the software pipeline runs mm1 TWO megas ahead of mm2 (so
        # mm1(2), which needs pair1, is emitted before mm2(0) and the tile
        # scheduler cannot hoist it in front of mm2(0) and stall the PE on
        # the pair1 DMA), leaving 3 qgr generations live at once.
        qgr_pool = ctx.enter_context(tc.tile_pool(name="qgrp", bufs=3))
        out_pool = ctx.enter_context(tc.tile_pool(name="outp", bufs=2))
        small_pool = ctx.enter_context(tc.tile_pool(name="smallp", bufs=2))
        qg_psum = ctx.enter_context(tc.tile_pool(name="qgps", bufs=2, space="PSUM"))
        msg_psum = ctx.enter_context(tc.tile_pool(name="msgps", bufs=6, space="PSUM"))

        # Warm tiles memset on GPSIMD (idle at start; DVE's queue is blocked
        # by its DGE-init TENSOR_LOAD until ~4.7us) so the junk matmuls can
        # start right at the Tensor preamble end (~6.6us).
        warm_a = const_pool.tile([128, 128], BF16)
        nc.gpsimd.memset(warm_a, 0.0)
        warm_b = const_pool.tile([128, 512], BF16)
        nc.gpsimd.memset(warm_b, 0.0)

        ones_f = const_pool.tile([128, 1], F32)
        nc.vector.memset(ones_f, 1.0)
        ones_g = const_pool.tile([128, 1], BF16)
        nc.vector.tensor_copy(ones_g, ones_f)

        # Weights on the second HWDGE ring (nc.scalar), k-sliced to match the
        # qzT pair-0 slices: singles at the head (earliest possible first
        # matmul) and at the tail (the last slice's ~1.3us completion-receipt
        # latency overlaps the previous slice's transfer instead of all
        # stacking behind one 256KB quarter).
        # bf16 slices are indexed in the k2..7 tensors' own 0..5 space
        K_SLICES = [(0, 2), (2, 4), (4, 5), (5, 6)]
        bwT8_sb = const_pool.tile([128, KF8, G], F8E4)
        nc.scalar.dma_start(out=bwT8_sb, in_=bwT8_d[:])
        bwT8b_sb = const_pool.tile([128, KF8, G], F8E4)
        nc.scalar.dma_start(out=bwT8b_sb, in_=bwT8b_d[:])
        bwT_sb = const_pool.tile([128, KD - KF8, G], BF16)
        # bf16 k2-3 weights are only read by megas >= NDR23 (~120us in):
        # load them last so they never gate the startup stream.
        for a, b in K_SLICES[1:] + K_SLICES[:1]:
            nc.scalar.dma_start(
                out=bwT_sb[:, a:b, :], in_=bwT_d[:, a:b, :]
            )
        # bw rides the sync ring BETWEEN pair0 and pair1: the two HWDGE
        # rings share the 16 SDMA engines, so anything queued early steals
        # bandwidth from the pair0 load that gates the first real matmuls.
        bw_sb = const_pool.tile([128, KG, D], BF16)

        # Junk matmuls bridge the DMA-bound load window so the PE_HAM clock
        # gate sees >=3.4us of sustained activity and the real matmul stream
        # starts warm. 3 N=128 ones (only need the small warm_a memset) lead,
        # then N=512 ones until the first qzT/bwT quarter lands (~12us).
        warm_ps = msg_psum.tile([128, 512], F32, name="warm_ps", tag="m_ps")
        for _ in range(3):
            nc.tensor.matmul(warm_ps[:, 0:128], warm_a, warm_a)
        for _ in range(11):
            nc.tensor.matmul(warm_ps, warm_a, warm_b)

        def load_qzT(j):
            # one tile = 2 megas (1024 rows). Pair 0 loads in k-slices so
            # the k-outer startup stream consumes them as they arrive; later
            # pairs are one DMA each (16KB contiguous runs per partition).
            qzT8 = in8_pool.tile([128, 2 * MEGA // 16, KF8, 16], F8E4, name="qzT8")
            qzT = in_pool.tile([128, KD - KF8, 2 * MEGA], BF16, name="qzT")
            qzT8b = None
            if j < NDR23 // 2:
                qzT8b = in8b_pool.tile(
                    [128, 2 * MEGA // 16, KF8, 16], F8E4, name="qzT8b"
                )
            if j == 0:
                # The whole kernel start is gated on the fp8 k01 slice's
                # completion sem (the PE is supply-fed and stall-free from
                # there on); it is the smallest piece (384KB with bwT8).
                # Finer slicing measurably hurts: every extra dma_start
                # costs ~0.6us of ring-issue serialization ahead of the
                # later quarters.
                nc.sync.dma_start(out=qzT8, in_=qzT8_d[:, 0])
                nc.sync.dma_start(out=qzT8b, in_=qzT8b_d[:, 0])
                # k2-3 bf16 of pair 0 is replaced by fp8: not loaded
                for a, b in K_SLICES[1:]:
                    nc.sync.dma_start(
                        out=qzT[:, a:b, :],
                        in_=qzT_d[:, 0, a:b, :],
                    )
            else:
                nc.sync.dma_start(out=qzT8, in_=qzT8_d[:, j])
                if qzT8b is not None:
                    nc.sync.dma_start(out=qzT8b, in_=qzT8b_d[:, j])
                    # bf16 k2-3 of this pair is replaced by fp8: skip it
                    nc.sync.dma_start(
                        out=qzT[:, 2:, :], in_=qzT_d[:, j, 2:, :]
                    )
                else:
                    nc.sync.dma_start(out=qzT, in_=qzT_d[:, j])
            return qzT8, qzT8b, qzT

        pairs = {}

        def ensure_load(j):
            if 0 <= j < NPAIR and j not in pairs:
                pairs[j] = load_qzT(j)

        def dr_mm1(qg_ps, qzT8, gc, c0, wsb=None, start=True):
            # fp8 DoubleRow: contracts one 256-d k-pair in one MM; the first
            # opens the psum accumulation group.
            nc.tensor.matmul(
                qg_ps,
                (wsb if wsb is not None else bwT8_sb)[:, :, gc * 128:(gc + 1) * 128],
                qzT8[:, c0 // 16:(c0 + MEGA) // 16, :, :].rearrange(
                    "p blk two c -> p two blk c"
                ),
                start=start,
                stop=False,
                perf_mode=DR,
            )

        def mm1(t):
            # qgT[gc] = sum_k bwT[:,k,gc].T @ qzT[:,k,cols(t)] -> relu (ACT)
            qzT8, qzT8b, qzT = pairs[t // 2]
            c0 = (t % 2) * MEGA
            kb0 = KF8 if t < NDR23 else 0
            qgr = qgr_pool.tile([128, KG, MEGA], BF16, name="qgr")
            def bf16_mm(qg_ps, gc, kb):
                nc.tensor.matmul(
                    qg_ps,
                    bwT_sb[:, kb, gc * 128:(gc + 1) * 128],
                    qzT[:, kb, c0:c0 + MEGA],
                    start=False,
                    stop=(kb == KD - KF8 - 1),
                )

            for gc in range(KG):
                qg_ps = qg_psum.tile([128, MEGA], F32, name="qg_ps")
                dr_mm1(qg_ps, qzT8, gc, c0)
                if t < NDR23:
                    # a bf16 matmul separates the two DR matmuls: a DR
                    # absorbs into a preceding bf16 stream (28ns issue gap
                    # measured) but NOT into a preceding DR (403ns) — the
                    # interleave saves ~300ns per group
                    bf16_mm(qg_ps, gc, kb0)
                    dr_mm1(qg_ps, qzT8b, gc, c0, wsb=bwT8b_sb, start=False)
                    for kb in range(kb0 + 1, KD - KF8):
                        bf16_mm(qg_ps, gc, kb)
                else:
                    for kb in range(kb0, KD - KF8):
                        bf16_mm(qg_ps, gc, kb)
                nc.scalar.activation(qgr[:, gc, :], qg_ps, AF.Relu)
            return qgr

        def mm1_pair0():
            # Startup-only mm1 for megas 0 and 1: k-OUTER accumulation with
            # all 8 gc psum groups open at once (2 qg-pool banks + 6 msg-pool
            # banks incl. the warmup slot — the msg pool is idle until mm2(0)
            # ~12us later). Each k-quarter of pair0 is consumed the moment it
            # lands, so the PE streams real work through the whole DMA-bound
            # startup window with zero gc-outer data stalls.
            qzT8, qzT8b, qzT = pairs[0]
            banks = [
                [
                    qg_psum.tile([128, MEGA], F32, name="qg_ps"),
                    qg_psum.tile([128, MEGA], F32, name="qg_ps"),
                    msg_psum.tile([128, MEGA], F32, name="m_ps"),
                    msg_psum.tile([128, MEGA], F32, name="m_ps"),
                ],
                [
                    msg_psum.tile([128, MEGA], F32, name="m_ps"),
                    msg_psum.tile([128, MEGA], F32, name="m_ps"),
                    msg_psum.tile([128, MEGA], F32, name="m_ps"),
                    msg_psum.tile([128, MEGA], F32, name="m_ps"),
                ],
            ]
            qgrs = [
                qgr_pool.tile([128, KG, MEGA], BF16, name="qgr"),
                qgr_pool.tile([128, KG, MEGA], BF16, name="qgr"),
            ]
            for mega in range(2):
                for gc in range(KG):
                    dr_mm1(banks[mega][gc], qzT8, gc, mega * MEGA)
            for mega in range(2):
                for gc in range(KG):
                    dr_mm1(
                        banks[mega][gc], qzT8b, gc, mega * MEGA,
                        wsb=bwT8b_sb, start=False,
                    )
            for kb in range(KF8, KD - KF8):
                for mega in range(2):
                    for gc in range(KG):
                        nc.tensor.matmul(
                            banks[mega][gc],
                            bwT_sb[:, kb, gc * 128:(gc + 1) * 128],
                            qzT[:, kb, mega * MEGA:(mega + 1) * MEGA],
                            start=False,
                            stop=(kb == KD - KF8 - 1),
                        )
            # Mega-0 relus split across ACT and DVE (max(x,0) — identical
            # rounding) so mm2(0)'s stationary is ready ~2 relu-times after
            # the last k7 matmul instead of 4; mega-1's relus run on ACT
            # while mm2(0) computes.
            nc.scalar.activation(qgrs[0][:, 0, :], banks[0][0], AF.Relu)
            nc.vector.tensor_scalar_max(qgrs[0][:, 1, :], banks[0][1], 0.0)
            nc.scalar.activation(qgrs[0][:, 2, :], banks[0][2], AF.Relu)
            nc.vector.tensor_scalar_max(qgrs[0][:, 3, :], banks[0][3], 0.0)
            for gc in range(KG):
                nc.scalar.activation(qgrs[1][:, gc, :], banks[1][gc], AF.Relu)
            return qgrs

        def mm2_block(t, qgr):
            msg_sb = out_pool.tile([128, NSUB, D], BF16, name="msg_sb")

            def mmgroup(s, h):
                m_ps = msg_psum.tile([128, 512], F32, name="m_ps")
                for gc in range(KG):
                    nc.tensor.matmul(
                        m_ps,
                        qgr[:, gc, s * 128:(s + 1) * 128],
                        bw_sb[:, gc, h * 512:(h + 1) * 512],
                        start=(gc == 0),
                        stop=(gc == KG - 1),
                    )
                return m_ps

            def drain(s, h, m_ps, sc_sb):
                # all drains on DVE: ACT only runs the relus, so a drain is
                # never queued behind the next mega's relus on ACT's strict
                # FIFO (that ordering stalled mm2 psum-slot reuse by ~3us)
                dst = msg_sb[:, s, h * 512:(h + 1) * 512]
                nc.vector.tensor_scalar_mul(dst, m_ps, sc_sb[:, s:s + 1])

            # rowsum over g, den-direct: DVE sums the 4 qgr chunks into
            # acc [128(g_low), p] (bf16, error ~1e-3 of den — negligible),
            # then 4 tiny N=1 matmuls acc_chunk.T @ ones produce den for
            # each 128-row sub ALREADY in per-partition column layout.
            # This replaces 4 N=512 rowsum MMs + 4 PE transposes + a DVE
            # copy (~1.2us of PE per mega) with ~0.35us of PE.
            ADD = mybir.AluOpType.add
            s1 = small_pool.tile([128, MEGA], BF16, name="acc_s1")
            nc.vector.scalar_tensor_tensor(s1, qgr[:, 0, :], 0.0, qgr[:, 1, :], ADD, ADD)
            s2 = small_pool.tile([128, MEGA], BF16, name="acc_s2")
            nc.vector.scalar_tensor_tensor(s2, qgr[:, 2, :], 0.0, qgr[:, 3, :], ADD, ADD)
            acc = small_pool.tile([128, MEGA], BF16, name="acc")
            nc.vector.scalar_tensor_tensor(acc, s1, 0.0, s2, ADD, ADD)

            pending = [(0, 0, mmgroup(0, 0))]
            pending.append((0, 1, mmgroup(0, 1)))
            pending.append((1, 0, mmgroup(1, 0)))

            # sc_ps lives in the msg pool: its slot's previous occupant was
            # drained a full mega ago. (In the qg pool it reused a slot whose
            # last reader is mm1(t+1)'s relu — a ~0.4us/mega PE stall.) The
            # tiny sc matmuls sit after three mm2 groups so mega 0 — whose
            # DVE rowsum can only start at relu time — has its acc ready.
            sc_ps = msg_psum.tile([128, 512], F32, name="sc_ps", tag="m_ps")
            for ss in range(NSUB):
                nc.tensor.matmul(
                    sc_ps[:, ss:ss + 1],
                    acc[:, ss * 128:(ss + 1) * 128],
                    ones_g,
                )

            sc_sb = small_pool.tile([128, NSUB], F32, name="sc_sb")
            nc.vector.tensor_scalar_max(sc_sb, sc_ps[:, 0:NSUB], EPS)
            nc.vector.reciprocal(sc_sb, sc_sb)

            pending.append((1, 1, mmgroup(1, 1)))
            for (ps_, hs_, mp_) in pending:
                drain(ps_, hs_, mp_, sc_sb)

            last = t == NMEGA - 1
            if last:
                for s in (0, 1):
                    nc.sync.dma_start(
                        out=msg_d[t * MEGA + s * 128:t * MEGA + (s + 1) * 128, :],
                        in_=msg_sb[:, s, :],
                    )
            for s in (2, 3):
                if last and s == 3:
                    # Final sub: store in 512-col halves so the h0 half
                    # (128KB) is already in flight while h1 computes, and
                    # put the very last drain on ACT (no later relus exist
                    # to queue behind) so it starts the moment the last MM
                    # retires. Trims ~1us off the last-MM -> last-byte tail.
                    m0 = mmgroup(s, 0)
                    drain(s, 0, m0, sc_sb)
                    nc.sync.dma_start(
                        out=msg_d[t * MEGA + s * 128:t * MEGA + (s + 1) * 128, 0:512],
                        in_=msg_sb[:, s, 0:512],
                    )
                    m1 = mmgroup(s, 1)
                    nc.scalar.mul(msg_sb[:, s, 512:1024], m1, sc_sb[:, s:s + 1])
                    nc.sync.dma_start(
                        out=msg_d[t * MEGA + s * 128:t * MEGA + (s + 1) * 128, 512:1024],
                        in_=msg_sb[:, s, 512:1024],
                    )
                    continue
                for h in (0, 1):
                    drain(s, h, mmgroup(s, h), sc_sb)
                if last:
                    # per-sub stores at the end: the final store is only
                    # 256KB, shrinking the post-compute tail
                    nc.sync.dma_start(
                        out=msg_d[t * MEGA + s * 128:t * MEGA + (s + 1) * 128, :],
                        in_=msg_sb[:, s, :],
                    )
            if not last:
                # one store per mega: fewer ring-issue slots and completion
                # semaphores (the teardown epilogue scales with DMA count)
                nc.sync.dma_start(
                    out=msg_d[t * MEGA:(t + 1) * MEGA, :].rearrange(
                        "(s p) d -> p s d", p=128
                    ),
                    in_=msg_sb,
                )

        ensure_load(0)
        # Sync-ring order pair0, pair1, bw, pair2: mm1(2) is the first PE
        # work after the pair-0 k-outer stream (the pipeline runs two megas
        # ahead), so pair1 must not queue behind the 1MB bw load; bw itself
        # is only needed by mm2(0), a full mega of PE work later.
        ensure_load(1)
        nc.sync.dma_start(
            out=bw_sb, in_=bw_d[:].rearrange("(gc p) d -> p gc d", p=128)
        )
        ensure_load(2)
        qgr_queue = list(mm1_pair0())
        for t in range(NMEGA):
            nxt = t + 2
            if nxt < NMEGA:
                if nxt % 2 == 0:
                    ensure_load(nxt // 2 + 1)
                qgr_queue.append(mm1(nxt))
            mm2_block(t, qgr_queue.pop(0))

    nc.compile()
    return nc


_NC_CACHE = None


def _get_nc():
    global _NC_CACHE
    if _NC_CACHE is None:
        _NC_CACHE = build_kernel()
    return _NC_CACHE


def kernel(qz: np.ndarray, binary_weight: np.ndarray) -> np.ndarray:
    import ml_dtypes

    bf16 = ml_dtypes.bfloat16
    qz = np.asarray(qz, dtype=np.float32)
    bw32 = np.asarray(binary_weight, dtype=np.float32)
    assert qz.shape == (B, C, P, D), qz.shape
    assert bw32.shape == (B, G, D), bw32.shape
    bw = bw32.astype(bf16)

    fp8 = ml_dtypes.float8_e4m3fn if hasattr(ml_dtypes, "float8_e4m3fn") else ml_dtypes.float8_e4m3

    nc = _get_nc()
    in_maps = []
    for i in range(N_CORES):
        # qzT[p, j, k, c] = qz[j*1024+c, k*128+p]: contiguous per
        # (partition, pair) so pair DMAs are long-run descriptors.
        # k-chunks 0..1 ship as fp8e4 (DoubleRow operand, quantized straight
        # from fp32), chunks 2..7 as bf16.
        qzt = qz[i].reshape(R, D).reshape(NPAIR, 2 * MEGA, KD, 128).transpose(3, 0, 2, 1)
        qzT8 = np.ascontiguousarray(
            qzt[:, :, :KF8, :].reshape(128, NPAIR, KF8, 2 * MEGA // 16, 16)
            .transpose(0, 1, 3, 2, 4).astype(fp8)
        )
        qzT8b = np.ascontiguousarray(
            qzt[:, :NDR23 // 2, KF8:2 * KF8, :]
            .reshape(128, NDR23 // 2, KF8, 2 * MEGA // 16, 16)
            .transpose(0, 1, 3, 2, 4).astype(fp8)
        )
        qzT = np.ascontiguousarray(qzt[:, :, KF8:, :].astype(bf16))
        # bwT[p, k, g] = bw[g, k*128+p]
        bwt = bw32[i].reshape(G, KD, 128).transpose(2, 1, 0)
        bwT8 = np.ascontiguousarray(bwt[:, :KF8].astype(fp8))
        bwT8b = np.ascontiguousarray(bwt[:, KF8:2 * KF8].astype(fp8))
        bwT = np.ascontiguousarray(bwt[:, KF8:].astype(bf16))
        in_maps.append(
            {"qzT": qzT, "qzT8": qzT8, "qzT8b": qzT8b, "bw": bw[i],
             "bwT": bwT, "bwT8": bwT8, "bwT8b": bwT8b}
        )
    res = run_bass_kernel_spmd(nc, in_maps, core_ids=list(range(N_CORES)))
    out = np.stack(
        [
            res.results[i]["msg"].astype(np.float32).reshape(C, P, D)
            for i in range(N_CORES)
        ],
        axis=0,
    )
    return out


# revision 48
# speedup vs baseline: 1.0141x; 1.0141x over previous
"""Trainium2 Bass kernel for DynamicPTTopicModeling.

Computes, per batch b (one batch per NeuronCore, 8 cores):
    qg  = relu(qz @ bw.T)            # [R=8192, G=512], contraction over d=1024
    den = max(sum_g qg, 1e-6)        # per-row L1 norm
    msg = (qg @ bw) / den            # [R, D=1024]

Sharding: batch b across the 8 NeuronCores, fully data-parallel (one batch
per core, no collectives). Host pre-swizzles qz/bw into partition-major
layouts; the output is stored bf16 and upcast on the host. Precision is
mixed against the 2e-2 tolerance: mm1's first 256 d-dims (k-chunks 0..1)
run as fp8e4 DoubleRow matmuls, the rest bf16 — measured end-to-end error
1.51e-2 (bf16-only: 3.2e-3), buying ~12us of PE stream + smaller DMAs.

The kernel is PE-streaming-bound: 1024 N=512 matmuls = ~221us at the warm
2.4 GHz floor (HW-verified 216 ns/MM cadence, zero mid-stream gaps). All
remaining tuning is startup/tail:
  - Host layouts are partition-major ([128, ...]) so every DMA has 2-16KB
    contiguous runs per partition: pair loads are one descriptor set of
    16KB runs, and the bwT ring no longer serves 1KB packets that would
    steal SDMA round-robin slots from the qzT stream at startup.
  - Pair 0 (megas 0+1) is consumed K-OUTER with all 8 PSUM banks open
    (2 qg-pool + 6 msg-pool incl. the warmup slot): each k-quarter
    (0.5MB qzT + 0.25MB bwT, ~2.1us at the ~375 GB/s HBM cap) is consumed
    the moment its completion sem fires, so the PE does real work through
    the whole DMA-bound window instead of idling on gc-outer dependencies.
  - Junk matmuls on gpsimd-memset tiles bridge t=7..12us (PE_HAM needs
    ~3.4us of activity to unthrottle 1.2->2.4 GHz) until the first quarter
    lands; 3 short N=128 ones lead so the bridge starts before the larger
    moving-operand memset completes.
  - Pair-0 relus split ACT/DVE (2+2 for mega 0) so mm2(0)'s stationary is
    ready ~1.4us earlier than 8 serial ACT relus would allow.
  - Steady state: software-pipelined one mega ahead (mm1(t+1) interleaved
    with rowsum(t)/mm2(t)); row-sums via DVE chunk-adds + 4 tiny N=1
    matmuls directly into per-partition column layout; scale applied
    during the PSUM drain copy. The final sub's store is split in halves
    with its last drain on ACT so the closing DMA is 128KB and in flight
    ~1us after the last matmul retires.
"""
from contextlib import ExitStack

import numpy as np

import concourse.bass as bass
import concourse.tile as tile
from concourse import bacc, mybir
from concourse.bass_utils import run_bass_kernel_spmd

F32 = mybir.dt.float32
BF16 = mybir.dt.bfloat16
F8E4 = mybir.dt.float8e4
AF = mybir.ActivationFunctionType
DR = mybir.MatmulPerfMode.DoubleRow
KF8 = 2              # k-chunks 0..1 (256 of 1024 d-dims) run fp8 DoubleRow
NDR23 = 12           # megas 0..11 additionally run k-chunks 2..3 as fp8 DR
                     # (error: 8 megas measured 1.837e-2, 12 megas predicted
                     # 1.982e-2 vs the 2e-2 gate — validated model, 0.4%
                     # accuracy; all 16 extrapolates to 2.12e-2 = over)

B, C, P, D, G = 8, 16, 512, 1024, 512
R = C * P            # 8192 rows per batch
MEGA = 512           # rows per mega-tile
NSUB = MEGA // 128   # 4
NMEGA = R // MEGA    # 16
NPAIR = NMEGA // 2   # 8 (qzT loads are 2 megas per tile)
KD = D // 128        # 8 d-chunks
KG = G // 128        # 4 g-chunks
EPS = 1e-6
N_CORES = 8


def build_kernel():
    nc = bacc.Bacc("TRN2", target_bir_lowering=False)
    # Host-pre-swizzled layouts: partition dim first so DMA runs are long
    # and contiguous. qzT[p, j, k, c] = qz[j*1024+c, k*128+p] (16KB/partition
    # per pair, 4KB per startup k-quarter). bwT[p, k, g] = bw[g, k*128+p]
    # (2KB per quarter).
    # d-contraction split: k-chunks 0..1 (256 dims) in fp8e4 consumed by one
    # DoubleRow matmul per (gc, mega); k-chunks 2..7 in bf16. The DR matmul
    # issues at ~408ns (HW streams the two k-halves as two passes; verified
    # identical for split and 16B-interleaved layouts), vs 432ns+overheads
    # for the two bf16 matmuls it replaces — the net ~12us win comes from
    # that plus 64 fewer issue slots and 12.5% smaller input DMAs in the
    # supply-bound startup. End-to-end error measured 1.51e-2 vs the 2e-2
    # gate (bf16-only is 3.2e-3).
    qzT_d = nc.dram_tensor(
        "qzT", [128, NPAIR, KD - KF8, 2 * MEGA], BF16, kind="ExternalInput"
    )
    # fp8 k0/k1 interleaved at 16-byte block granularity: per partition
    # [k0 c0-15][k1 c0-15][k0 c16-31]... so the DoubleRow moving AP's second
    # dim is the k-pair at stride 16 (the verifier's Num=2, Size%16==0 form
    # at the SBUF line size) instead of stride 1024 (measured 408ns/MM,
    # i.e. two full passes).
    qzT8_d = nc.dram_tensor(
        "qzT8", [128, NPAIR, 2 * MEGA // 16, KF8, 16], F8E4, kind="ExternalInput"
    )
    qzT8b_d = nc.dram_tensor(
        "qzT8b", [128, NDR23 // 2, 2 * MEGA // 16, KF8, 16], F8E4, kind="ExternalInput"
    )
    bw_d = nc.dram_tensor("bw", [G, D], BF16, kind="ExternalInput")
    bwT_d = nc.dram_tensor("bwT", [128, KD - KF8, G], BF16, kind="ExternalInput")
    bwT8_d = nc.dram_tensor("bwT8", [128, KF8, G], F8E4, kind="ExternalInput")
    bwT8b_d = nc.dram_tensor("bwT8b", [128, KF8, G], F8E4, kind="ExternalInput")
    msg_d = nc.dram_tensor("msg", [R, D], BF16, kind="ExternalOutput")

    with tile.TileContext(nc) as tc, ExitStack() as ctx:
        const_pool = ctx.enter_context(tc.tile_pool(name="const", bufs=1))
        in_pool = ctx.enter_context(tc.tile_pool(name="inp", bufs=3))
        in8_pool = ctx.enter_context(tc.tile_pool(name="inp8", bufs=3))
        in8b_pool = ctx.enter_context(tc.tile_pool(name="inp8b", bufs=3))
        # bufs=3: the software pipeline runs mm1 TWO megas ahead of mm2 (so
        # mm1(2), which needs pair1, is emitted before mm2(0) and the tile
        # scheduler cannot hoist it in front of mm2(0) and stall the PE on
        # the pair1 DMA), leaving 3 qgr generations live at once.
        qgr_pool = ctx.enter_context(tc.tile_pool(name="qgrp", bufs=3))
        out_pool = ctx.enter_context(tc.tile_pool(name="outp", bufs=2))
        small_pool = ctx.enter_context(tc.tile_pool(name="smallp", bufs=2))
        qg_psum = ctx.enter_context(tc.tile_pool(name="qgps", bufs=2, space="PSUM"))
        msg_psum = ctx.enter_context(tc.tile_pool(name="msgps", bufs=6, space="PSUM"))

        # Warm tiles memset on GPSIMD (idle at start; DVE's queue is blocked
        # by its DGE-init TENSOR_LOAD until ~4.7us) so the junk matmuls can
        # start right at the Tensor preamble end (~6.6us).
        warm_a = const_pool.tile([128, 128], BF16)
        nc.gpsimd.memset(warm_a, 0.0)
        warm_b = const_pool.tile([128, 512], BF16)
        nc.gpsimd.memset(warm_b, 0.0)

        ones_f = const_pool.tile([128, 1], F32)
        nc.vector.memset(ones_f, 1.0)
        ones_g = const_pool.tile([128, 1], BF16)
        nc.vector.tensor_copy(ones_g, ones_f)

        # Weights on the second HWDGE ring (nc.scalar), k-sliced to match the
        # qzT pair-0 slices: singles at the head (earliest possible first
        # matmul) and at the tail (the last slice's ~1.3us completion-receipt
        # latency overlaps the previous slice's transfer instead of all
        # stacking behind one 256KB quarter).
        # bf16 slices are indexed in the k2..7 tensors' own 0..5 space
        K_SLICES = [(0, 2), (2, 4), (4, 5), (5, 6)]
        bwT8_sb = const_pool.tile([128, KF8, G], F8E4)
        nc.scalar.dma_start(out=bwT8_sb, in_=bwT8_d[:])
        bwT8b_sb = const_pool.tile([128, KF8, G], F8E4)
        nc.scalar.dma_start(out=bwT8b_sb, in_=bwT8b_d[:])
        bwT_sb = const_pool.tile([128, KD - KF8, G], BF16)
        # bf16 k2-3 weights are only read by megas >= NDR23 (~120us in):
        # load them last so they never gate the startup stream.
        for a, b in K_SLICES[1:] + K_SLICES[:1]:
            nc.scalar.dma_start(
                out=bwT_sb[:, a:b, :], in_=bwT_d[:, a:b, :]
            )
        # bw rides the sync ring BETWEEN pair0 and pair1: the two HWDGE
        # rings share the 16 SDMA engines, so anything queued early steals
        # bandwidth from the pair0 load that gates the first real matmuls.
        bw_sb = const_pool.tile([128, KG, D], BF16)

        # Junk matmuls bridge the DMA-bound load window so the PE_HAM clock
        # gate sees >=3.4us of sustained activity and the real matmul stream
        # starts warm. 3 N=128 ones (only need the small warm_a memset) lead,
        # then N=512 ones until the first qzT/bwT quarter lands (~12us).
        warm_ps = msg_psum.tile([128, 512], F32, name="warm_ps", tag="m_ps")
        for _ in range(3):
            nc.tensor.matmul(warm_ps[:, 0:128], warm_a, warm_a)
        for _ in range(11):
            nc.tensor.matmul(warm_ps, warm_a, warm_b)

        def load_qzT(j):
            # one tile = 2 megas (1024 rows). Pair 0 loads in k-slices so
            # the k-outer startup stream consumes them as they arrive; later
            # pairs are one DMA each (16KB contiguous runs per partition).
            qzT8 = in8_pool.tile([128, 2 * MEGA // 16, KF8, 16], F8E4, name="qzT8")
            qzT = in_pool.tile([128, KD - KF8, 2 * MEGA], BF16, name="qzT")
            qzT8b = None
            if j < NDR23 // 2:
                qzT8b = in8b_pool.tile(
                    [128, 2 * MEGA // 16, KF8, 16], F8E4, name="qzT8b"
                )
            if j == 0:
                # The whole kernel start is gated on the fp8 k01 slice's
                # completion sem (the PE is supply-fed and stall-free from
                # there on); it is the smallest piece (384KB with bwT8).
                # Finer slicing measurably hurts: every extra dma_start
                # costs ~0.6us of ring-issue serialization ahead of the
                # later quarters.
                nc.sync.dma_start(out=qzT8, in_=qzT8_d[:, 0])
                nc.sync.dma_start(out=qzT8b, in_=qzT8b_d[:, 0])
                # k2-3 bf16 of pair 0 is replaced by fp8: not loaded
                for a, b in K_SLICES[1:]:
                    nc.sync.dma_start(
                        out=qzT[:, a:b, :],
                        in_=qzT_d[:, 0, a:b, :],
                    )
            else:
                nc.sync.dma_start(out=qzT8, in_=qzT8_d[:, j])
                if qzT8b is not None:
                    nc.sync.dma_start(out=qzT8b, in_=qzT8b_d[:, j])
                    # bf16 k2-3 of this pair is replaced by fp8: skip it
                    nc.sync.dma_start(
                        out=qzT[:, 2:, :], in_=qzT_d[:, j, 2:, :]
                    )
                else:
                    nc.sync.dma_start(out=qzT, in_=qzT_d[:, j])
            return qzT8, qzT8b, qzT

        pairs = {}

        def ensure_load(j):
            if 0 <= j < NPAIR and j not in pairs:
                pairs[j] = load_qzT(j)

        def dr_mm1(qg_ps, qzT8, gc, c0, wsb=None, start=True):
            # fp8 DoubleRow: contracts one 256-d k-pair in one MM; the first
            # opens the psum accumulation group.
            nc.tensor.matmul(
                qg_ps,
                (wsb if wsb is not None else bwT8_sb)[:, :, gc * 128:(gc + 1) * 128],
                qzT8[:, c0 // 16:(c0 + MEGA) // 16, :, :].rearrange(
                    "p blk two c -> p two blk c"
                ),
                start=start,
                stop=False,
                perf_mode=DR,
            )

        def mm1(t):
            # qgT[gc] = sum_k bwT[:,k,gc].T @ qzT[:,k,cols(t)] -> relu (ACT)
            qzT8, qzT8b, qzT = pairs[t // 2]
            c0 = (t % 2) * MEGA
            kb0 = KF8 if t < NDR23 else 0
            qgr = qgr_pool.tile([128, KG, MEGA], BF16, name="qgr")
            for gc in range(KG):
                qg_ps = qg_psum.tile([128, MEGA], F32, name="qg_ps")
                dr_mm1(qg_ps, qzT8, gc, c0)
                if t < NDR23:
                    dr_mm1(qg_ps, qzT8b, gc, c0, wsb=bwT8b_sb, start=False)
                for kb in range(kb0, KD - KF8):
                    nc.tensor.matmul(
                        qg_ps,
                        bwT_sb[:, kb, gc * 128:(gc + 1) * 128],
                        qzT[:, kb, c0:c0 + MEGA],
                        start=False,
                        stop=(kb == KD - KF8 - 1),
                    )
                nc.scalar.activation(qgr[:, gc, :], qg_ps, AF.Relu)
            return qgr

        def mm1_pair0():
            # Startup-only mm1 for megas 0 and 1: k-OUTER accumulation with
            # all 8 gc psum groups open at once (2 qg-pool banks + 6 msg-pool
            # banks incl. the warmup slot — the msg pool is idle until mm2(0)
            # ~12us later). Each k-quarter of pair0 is consumed the moment it
            # lands, so the PE streams real work through the whole DMA-bound
            # startup window with zero gc-outer data stalls.
            qzT8, qzT8b, qzT = pairs[0]
            banks = [
                [
                    qg_psum.tile([128, MEGA], F32, name="qg_ps"),
                    qg_psum.tile([128, MEGA], F32, name="qg_ps"),
                    msg_psum.tile([128, MEGA], F32, name="m_ps"),
                    msg_psum.tile([128, MEGA], F32, name="m_ps"),
                ],
                [
                    msg_psum.tile([128, MEGA], F32, name="m_ps"),
                    msg_psum.tile([128, MEGA], F32, name="m_ps"),
                    msg_psum.tile([128, MEGA], F32, name="m_ps"),
                    msg_psum.tile([128, MEGA], F32, name="m_ps"),
                ],
            ]
            qgrs = [
                qgr_pool.tile([128, KG, MEGA], BF16, name="qgr"),
                qgr_pool.tile([128, KG, MEGA], BF16, name="qgr"),
            ]
            for mega in range(2):
                for gc in range(KG):
                    dr_mm1(banks[mega][gc], qzT8, gc, mega * MEGA)
            for mega in range(2):
                for gc in range(KG):
                    dr_mm1(
                        banks[mega][gc], qzT8b, gc, mega * MEGA,
                        wsb=bwT8b_sb, start=False,
                    )
            for kb in range(KF8, KD - KF8):
                for mega in range(2):
                    for gc in range(KG):
                        nc.tensor.matmul(
                            banks[mega][gc],
                            bwT_sb[:, kb, gc * 128:(gc + 1) * 128],
                            qzT[:, kb, mega * MEGA:(mega + 1) * MEGA],
                            start=False,
                            stop=(kb == KD - KF8 - 1),
                        )
            # Mega-0 relus split across ACT and DVE (max(x,0) — identical
            # rounding) so mm2(0)'s stationary is ready ~2 relu-times after
            # the last k7 matmul instead of 4; mega-1's relus run on ACT
            # while mm2(0) computes.
            nc.scalar.activation(qgrs[0][:, 0, :], banks[0][0], AF.Relu)
            nc.vector.tensor_scalar_max(qgrs[0][:, 1, :], banks[0][1], 0.0)
            nc.scalar.activation(qgrs[0][:, 2, :], banks[0][2], AF.Relu)
            nc.vector.tensor_scalar_max(qgrs[0][:, 3, :], banks[0][3], 0.0)
            for gc in range(KG):
                nc.scalar.activation(qgrs[1][:, gc, :], banks[1][gc], AF.Relu)
            return qgrs

        def mm2_block(t, qgr):
            msg_sb = out_pool.tile([128, NSUB, D], BF16, name="msg_sb")

            def mmgroup(s, h):
                m_ps = msg_psum.tile([128, 512], F32, name="m_ps")
                for gc in range(KG):
                    nc.tensor.matmul(
                        m_ps,
                        qgr[:, gc, s * 128:(s + 1) * 128],
                        bw_sb[:, gc, h * 512:(h + 1) * 512],
                        start=(gc == 0),
                        stop=(gc == KG - 1),
                    )
                return m_ps

            def drain(s, h, m_ps, sc_sb):
                # all drains on DVE: ACT only runs the relus, so a drain is
                # never queued behind the next mega's relus on ACT's strict
                # FIFO (that ordering stalled mm2 psum-slot reuse by ~3us)
                dst = msg_sb[:, s, h * 512:(h + 1) * 512]
                nc.vector.tensor_scalar_mul(dst, m_ps, sc_sb[:, s:s + 1])

            # rowsum over g, den-direct: DVE sums the 4 qgr chunks into
            # acc [128(g_low), p] (bf16, error ~1e-3 of den — negligible),
            # then 4 tiny N=1 matmuls acc_chunk.T @ ones produce den for
            # each 128-row sub ALREADY in per-partition column layout.
            # This replaces 4 N=512 rowsum MMs + 4 PE transposes + a DVE
            # copy (~1.2us of PE per mega) with ~0.35us of PE.
            ADD = mybir.AluOpType.add
            s1 = small_pool.tile([128, MEGA], BF16, name="acc_s1")
            nc.vector.scalar_tensor_tensor(s1, qgr[:, 0, :], 0.0, qgr[:, 1, :], ADD, ADD)
            s2 = small_pool.tile([128, MEGA], BF16, name="acc_s2")
            nc.vector.scalar_tensor_tensor(s2, qgr[:, 2, :], 0.0, qgr[:, 3, :], ADD, ADD)
            acc = small_pool.tile([128, MEGA], BF16, name="acc")
            nc.vector.scalar_tensor_tensor(acc, s1, 0.0, s2, ADD, ADD)

            pending = [(0, 0, mmgroup(0, 0))]
            pending.append((0, 1, mmgroup(0, 1)))
            pending.append((1, 0, mmgroup(1, 0)))

            # sc_ps lives in the msg pool: its slot's previous occupant was
            # drained a full mega ago. (In the qg pool it reused a slot whose
            # last reader is mm1(t+1)'s relu — a ~0.4us/mega PE stall.) The
            # tiny sc matmuls sit after three mm2 groups so mega 0 — whose
            # DVE rowsum can only start at relu time — has its acc ready.
            sc_ps = msg_psum.tile([128, 512], F32, name="sc_ps", tag="m_ps")
            for ss in range(NSUB):
                nc.tensor.matmul(
                    sc_ps[:, ss:ss + 1],
                    acc[:, ss * 128:(ss + 1) * 128],
                    ones_g,
                )

            sc_sb = small_pool.tile([128, NSUB], F32, name="sc_sb")
            nc.vector.tensor_scalar_max(sc_sb, sc_ps[:, 0:NSUB], EPS)
            nc.vector.reciprocal(sc_sb, sc_sb)

            pending.append((1, 1, mmgroup(1, 1)))
            for (ps_, hs_, mp_) in pending:
                drain(ps_, hs_, mp_, sc_sb)

            last = t == NMEGA - 1
            if last:
                for s in (0, 1):
                    nc.sync.dma_start(
                        out=msg_d[t * MEGA + s * 128:t * MEGA + (s + 1) * 128, :],
                        in_=msg_sb[:, s, :],
                    )
            for s in (2, 3):
                if last and s == 3:
                    # Final sub: store in 512-col halves so the h0 half
                    # (128KB) is already in flight while h1 computes, and
                    # put the very last drain on ACT (no later relus exist
                    # to queue behind) so it starts the moment the last MM
                    # retires. Trims ~1us off the last-MM -> last-byte tail.
                    m0 = mmgroup(s, 0)
                    drain(s, 0, m0, sc_sb)
                    nc.sync.dma_start(
                        out=msg_d[t * MEGA + s * 128:t * MEGA + (s + 1) * 128, 0:512],
                        in_=msg_sb[:, s, 0:512],
                    )
                    m1 = mmgroup(s, 1)
                    nc.scalar.mul(msg_sb[:, s, 512:1024], m1, sc_sb[:, s:s + 1])
                    nc.sync.dma_start(
                        out=msg_d[t * MEGA + s * 128:t * MEGA + (s + 1) * 128, 512:1024],
                        in_=msg_sb[:, s, 512:1024],
                    )
                    continue
                for h in (0, 1):
                    drain(s, h, mmgroup(s, h), sc_sb)
                if last:
                    # per-sub stores at the end: the final store is only
                    # 256KB, shrinking the post-compute tail
                    nc.sync.dma_start(
                        out=msg_d[t * MEGA + s * 128:t * MEGA + (s + 1) * 128, :],
                        in_=msg_sb[:, s, :],
                    )
            if not last:
                # one store per mega: fewer ring-issue slots and completion
                # semaphores (the teardown epilogue scales with DMA count)
                nc.sync.dma_start(
                    out=msg_d[t * MEGA:(t + 1) * MEGA, :].rearrange(
                        "(s p) d -> p s d", p=128
                    ),
                    in_=msg_sb,
                )

        ensure_load(0)
        # Sync-ring order pair0, pair1, bw, pair2: mm1(2) is the first PE
        # work after the pair-0 k-outer stream (the pipeline runs two megas
        # ahead), so pair1 must not queue behind the 1MB bw load; bw itself
        # is only needed by mm2(0), a full mega of PE work later.
        ensure_load(1)
        nc.sync.dma_start(
            out=bw_sb, in_=bw_d[:].rearrange("(gc p) d -> p gc d", p=128)
        )
        ensure_load(2)
        qgr_queue = list(mm1_pair0())
        for t in range(NMEGA):
            nxt = t + 2
            if nxt < NMEGA:
                if nxt % 2 == 0:
                    ensure_load(nxt // 2 + 1)
                qgr_queue.append(mm1(nxt))
            mm2_block(t, qgr_queue.pop(0))

    nc.compile()
    return nc


_NC_CACHE = None


def _get_nc():
    global _NC_CACHE
    if _NC_CACHE is None:
        _NC_CACHE = build_kernel()
    return _NC_CACHE


def kernel(qz: np.ndarray, binary_weight: np.ndarray) -> np.ndarray:
    import ml_dtypes

    bf16 = ml_dtypes.bfloat16
    qz = np.asarray(qz, dtype=np.float32)
    bw32 = np.asarray(binary_weight, dtype=np.float32)
    assert qz.shape == (B, C, P, D), qz.shape
    assert bw32.shape == (B, G, D), bw32.shape
    bw = bw32.astype(bf16)

    fp8 = ml_dtypes.float8_e4m3fn if hasattr(ml_dtypes, "float8_e4m3fn") else ml_dtypes.float8_e4m3

    nc = _get_nc()
    in_maps = []
    for i in range(N_CORES):
        # qzT[p, j, k, c] = qz[j*1024+c, k*128+p]: contiguous per
        # (partition, pair) so pair DMAs are long-run descriptors.
        # k-chunks 0..1 ship as fp8e4 (DoubleRow operand, quantized straight
        # from fp32), chunks 2..7 as bf16.
        qzt = qz[i].reshape(R, D).reshape(NPAIR, 2 * MEGA, KD, 128).transpose(3, 0, 2, 1)
        qzT8 = np.ascontiguousarray(
            qzt[:, :, :KF8, :].reshape(128, NPAIR, KF8, 2 * MEGA // 16, 16)
            .transpose(0, 1, 3, 2, 4).astype(fp8)
        )
        qzT8b = np.ascontiguousarray(
            qzt[:, :NDR23 // 2, KF8:2 * KF8, :]
            .reshape(128, NDR23 // 2, KF8, 2 * MEGA // 16, 16)
            .transpose(0, 1, 3, 2, 4).astype(fp8)
        )
        qzT = np.ascontiguousarray(qzt[:, :, KF8:, :].astype(bf16))
        # bwT[p, k, g] = bw[g, k*128+p]
        bwt = bw32[i].reshape(G, KD, 128).transpose(2, 1, 0)
        bwT8 = np.ascontiguousarray(bwt[:, :KF8].astype(fp8))
        bwT8b = np.ascontiguousarray(bwt[:, KF8:2 * KF8].astype(fp8))
        bwT = np.ascontiguousarray(bwt[:, KF8:].astype(bf16))
        in_maps.append(
            {"qzT": qzT, "qzT8": qzT8, "qzT8b": qzT8b, "bw": bw[i],
             "bwT": bwT, "bwT8": bwT8, "bwT8b": bwT8b}
        )
    res = run_bass_kernel_spmd(nc, in_maps, core_ids=list(range(N_CORES)))
    out = np.stack(
        [
            res.results[i]["msg"].astype(np.float32).reshape(C, P, D)
            for i in range(N_CORES)
        ],
        axis=0,
    )
    return out


# revision 49
# speedup vs baseline: 1.0154x; 1.0013x over previous
"""Trainium2 Bass kernel for DynamicPTTopicModeling.

Computes, per batch b (one batch per NeuronCore, 8 cores):
    qg  = relu(qz @ bw.T)            # [R=8192, G=512], contraction over d=1024
    den = max(sum_g qg, 1e-6)        # per-row L1 norm
    msg = (qg @ bw) / den            # [R, D=1024]

Sharding: batch b across the 8 NeuronCores, fully data-parallel (one batch
per core, no collectives). Host pre-swizzles qz/bw into partition-major
layouts; the output is stored bf16 and upcast on the host. Precision is
mixed against the 2e-2 tolerance: mm1's first 256 d-dims (k-chunks 0..1)
run as fp8e4 DoubleRow matmuls, the rest bf16 — measured end-to-end error
1.51e-2 (bf16-only: 3.2e-3), buying ~12us of PE stream + smaller DMAs.

The kernel is PE-streaming-bound: 1024 N=512 matmuls = ~221us at the warm
2.4 GHz floor (HW-verified 216 ns/MM cadence, zero mid-stream gaps). All
remaining tuning is startup/tail:
  - Host layouts are partition-major ([128, ...]) so every DMA has 2-16KB
    contiguous runs per partition: pair loads are one descriptor set of
    16KB runs, and the bwT ring no longer serves 1KB packets that would
    steal SDMA round-robin slots from the qzT stream at startup.
  - Pair 0 (megas 0+1) is consumed K-OUTER with all 8 PSUM banks open
    (2 qg-pool + 6 msg-pool incl. the warmup slot): each k-quarter
    (0.5MB qzT + 0.25MB bwT, ~2.1us at the ~375 GB/s HBM cap) is consumed
    the moment its completion sem fires, so the PE does real work through
    the whole DMA-bound window instead of idling on gc-outer dependencies.
  - Junk matmuls on gpsimd-memset tiles bridge t=7..12us (PE_HAM needs
    ~3.4us of activity to unthrottle 1.2->2.4 GHz) until the first quarter
    lands; 3 short N=128 ones lead so the bridge starts before the larger
    moving-operand memset completes.
  - Pair-0 relus split ACT/DVE (2+2 for mega 0) so mm2(0)'s stationary is
    ready ~1.4us earlier than 8 serial ACT relus would allow.
  - Steady state: software-pipelined one mega ahead (mm1(t+1) interleaved
    with rowsum(t)/mm2(t)); row-sums via DVE chunk-adds + 4 tiny N=1
    matmuls directly into per-partition column layout; scale applied
    during the PSUM drain copy. The final sub's store is split in halves
    with its last drain on ACT so the closing DMA is 128KB and in flight
    ~1us after the last matmul retires.
"""
from contextlib import ExitStack

import numpy as np

import concourse.bass as bass
import concourse.tile as tile
from concourse import bacc, mybir
from concourse.bass_utils import run_bass_kernel_spmd

F32 = mybir.dt.float32
BF16 = mybir.dt.bfloat16
F8E4 = mybir.dt.float8e4
AF = mybir.ActivationFunctionType
DR = mybir.MatmulPerfMode.DoubleRow
KF8 = 2              # k-chunks 0..1 (256 of 1024 d-dims) run fp8 DoubleRow
NDR23 = 12           # megas 0..11 additionally run k-chunks 2..3 as fp8 DR
                     # (error: 8 megas measured 1.837e-2, 12 megas predicted
                     # 1.982e-2 vs the 2e-2 gate — validated model, 0.4%
                     # accuracy; all 16 extrapolates to 2.12e-2 = over)

B, C, P, D, G = 8, 16, 512, 1024, 512
R = C * P            # 8192 rows per batch
MEGA = 512           # rows per mega-tile
NSUB = MEGA // 128   # 4
NMEGA = R // MEGA    # 16
NPAIR = NMEGA // 2   # 8 (qzT loads are 2 megas per tile)
KD = D // 128        # 8 d-chunks
KG = G // 128        # 4 g-chunks
EPS = 1e-6
N_CORES = 8


def build_kernel():
    nc = bacc.Bacc("TRN2", target_bir_lowering=False)
    # Host-pre-swizzled layouts: partition dim first so DMA runs are long
    # and contiguous. qzT[p, j, k, c] = qz[j*1024+c, k*128+p] (16KB/partition
    # per pair, 4KB per startup k-quarter). bwT[p, k, g] = bw[g, k*128+p]
    # (2KB per quarter).
    # d-contraction split: k-chunks 0..1 (256 dims) in fp8e4 consumed by one
    # DoubleRow matmul per (gc, mega); k-chunks 2..7 in bf16. The DR matmul
    # issues at ~408ns (HW streams the two k-halves as two passes; verified
    # identical for split and 16B-interleaved layouts), vs 432ns+overheads
    # for the two bf16 matmuls it replaces — the net ~12us win comes from
    # that plus 64 fewer issue slots and 12.5% smaller input DMAs in the
    # supply-bound startup. End-to-end error measured 1.51e-2 vs the 2e-2
    # gate (bf16-only is 3.2e-3).
    qzT_d = nc.dram_tensor(
        "qzT", [128, NPAIR, KD - KF8, 2 * MEGA], BF16, kind="ExternalInput"
    )
    # fp8 k0/k1 interleaved at 16-byte block granularity: per partition
    # [k0 c0-15][k1 c0-15][k0 c16-31]... so the DoubleRow moving AP's second
    # dim is the k-pair at stride 16 (the verifier's Num=2, Size%16==0 form
    # at the SBUF line size) instead of stride 1024 (measured 408ns/MM,
    # i.e. two full passes).
    qzT8_d = nc.dram_tensor(
        "qzT8", [128, NPAIR, 2 * MEGA // 16, KF8, 16], F8E4, kind="ExternalInput"
    )
    qzT8b_d = nc.dram_tensor(
        "qzT8b", [128, NDR23 // 2, 2 * MEGA // 16, KF8, 16], F8E4, kind="ExternalInput"
    )
    bw_d = nc.dram_tensor("bw", [G, D], BF16, kind="ExternalInput")
    bwT_d = nc.dram_tensor("bwT", [128, KD - KF8, G], BF16, kind="ExternalInput")
    bwT8_d = nc.dram_tensor("bwT8", [128, KF8, G], F8E4, kind="ExternalInput")
    bwT8b_d = nc.dram_tensor("bwT8b", [128, KF8, G], F8E4, kind="ExternalInput")
    msg_d = nc.dram_tensor("msg", [R, D], BF16, kind="ExternalOutput")

    with tile.TileContext(nc) as tc, ExitStack() as ctx:
        const_pool = ctx.enter_context(tc.tile_pool(name="const", bufs=1))
        in_pool = ctx.enter_context(tc.tile_pool(name="inp", bufs=3))
        in8_pool = ctx.enter_context(tc.tile_pool(name="inp8", bufs=3))
        in8b_pool = ctx.enter_context(tc.tile_pool(name="inp8b", bufs=3))
        # bufs=3: the software pipeline runs mm1 TWO megas ahead of mm2 (so
        # mm1(2), which needs pair1, is emitted before mm2(0) and the tile
        # scheduler cannot hoist it in front of mm2(0) and stall the PE on
        # the pair1 DMA), leaving 3 qgr generations live at once.
        qgr_pool = ctx.enter_context(tc.tile_pool(name="qgrp", bufs=3))
        out_pool = ctx.enter_context(tc.tile_pool(name="outp", bufs=2))
        small_pool = ctx.enter_context(tc.tile_pool(name="smallp", bufs=2))
        qg_psum = ctx.enter_context(tc.tile_pool(name="qgps", bufs=2, space="PSUM"))
        msg_psum = ctx.enter_context(tc.tile_pool(name="msgps", bufs=6, space="PSUM"))

        # Warm tiles memset on GPSIMD (idle at start; DVE's queue is blocked
        # by its DGE-init TENSOR_LOAD until ~4.7us) so the junk matmuls can
        # start right at the Tensor preamble end (~6.6us).
        warm_a = const_pool.tile([128, 128], BF16)
        nc.gpsimd.memset(warm_a, 0.0)
        warm_b = const_pool.tile([128, 512], BF16)
        nc.gpsimd.memset(warm_b, 0.0)

        ones_f = const_pool.tile([128, 1], F32)
        nc.vector.memset(ones_f, 1.0)
        ones_g = const_pool.tile([128, 1], BF16)
        nc.vector.tensor_copy(ones_g, ones_f)

        # Weights on the second HWDGE ring (nc.scalar), k-sliced to match the
        # qzT pair-0 slices: singles at the head (earliest possible first
        # matmul) and at the tail (the last slice's ~1.3us completion-receipt
        # latency overlaps the previous slice's transfer instead of all
        # stacking behind one 256KB quarter).
        # bf16 slices are indexed in the k2..7 tensors' own 0..5 space
        K_SLICES = [(0, 2), (2, 4), (4, 5), (5, 6)]
        bwT8_sb = const_pool.tile([128, KF8, G], F8E4)
        nc.scalar.dma_start(out=bwT8_sb, in_=bwT8_d[:])
        bwT8b_sb = const_pool.tile([128, KF8, G], F8E4)
        nc.scalar.dma_start(out=bwT8b_sb, in_=bwT8b_d[:])
        bwT_sb = const_pool.tile([128, KD - KF8, G], BF16)
        # bf16 k2-3 weights are only read by megas >= NDR23 (~120us in):
        # load them last so they never gate the startup stream.
        for a, b in K_SLICES[1:] + K_SLICES[:1]:
            nc.scalar.dma_start(
                out=bwT_sb[:, a:b, :], in_=bwT_d[:, a:b, :]
            )
        # bw rides the sync ring BETWEEN pair0 and pair1: the two HWDGE
        # rings share the 16 SDMA engines, so anything queued early steals
        # bandwidth from the pair0 load that gates the first real matmuls.
        bw_sb = const_pool.tile([128, KG, D], BF16)

        # Junk matmuls bridge the DMA-bound load window so the PE_HAM clock
        # gate sees >=3.4us of sustained activity and the real matmul stream
        # starts warm. 3 N=128 ones (only need the small warm_a memset) lead,
        # then N=512 ones until the first qzT/bwT quarter lands (~12us).
        warm_ps = msg_psum.tile([128, 512], F32, name="warm_ps", tag="m_ps")
        for _ in range(3):
            nc.tensor.matmul(warm_ps[:, 0:128], warm_a, warm_a)
        for _ in range(9):
            nc.tensor.matmul(warm_ps, warm_a, warm_b)

        def load_qzT(j):
            # one tile = 2 megas (1024 rows). Pair 0 loads in k-slices so
            # the k-outer startup stream consumes them as they arrive; later
            # pairs are one DMA each (16KB contiguous runs per partition).
            qzT8 = in8_pool.tile([128, 2 * MEGA // 16, KF8, 16], F8E4, name="qzT8")
            qzT = in_pool.tile([128, KD - KF8, 2 * MEGA], BF16, name="qzT")
            qzT8b = None
            if j < NDR23 // 2:
                qzT8b = in8b_pool.tile(
                    [128, 2 * MEGA // 16, KF8, 16], F8E4, name="qzT8b"
                )
            if j == 0:
                # The whole kernel start is gated on the fp8 k01 slice's
                # completion sem (the PE is supply-fed and stall-free from
                # there on); it is the smallest piece (384KB with bwT8).
                # Finer slicing measurably hurts: every extra dma_start
                # costs ~0.6us of ring-issue serialization ahead of the
                # later quarters.
                nc.sync.dma_start(out=qzT8, in_=qzT8_d[:, 0])
                nc.sync.dma_start(out=qzT8b, in_=qzT8b_d[:, 0])
                # k2-3 bf16 of pair 0 is replaced by fp8: not loaded
                for a, b in K_SLICES[1:]:
                    nc.sync.dma_start(
                        out=qzT[:, a:b, :],
                        in_=qzT_d[:, 0, a:b, :],
                    )
            else:
                nc.sync.dma_start(out=qzT8, in_=qzT8_d[:, j])
                if qzT8b is not None:
                    nc.sync.dma_start(out=qzT8b, in_=qzT8b_d[:, j])
                    # bf16 k2-3 of this pair is replaced by fp8: skip it
                    nc.sync.dma_start(
                        out=qzT[:, 2:, :], in_=qzT_d[:, j, 2:, :]
                    )
                else:
                    nc.sync.dma_start(out=qzT, in_=qzT_d[:, j])
            return qzT8, qzT8b, qzT

        pairs = {}

        def ensure_load(j):
            if 0 <= j < NPAIR and j not in pairs:
                pairs[j] = load_qzT(j)

        def dr_mm1(qg_ps, qzT8, gc, c0, wsb=None, start=True):
            # fp8 DoubleRow: contracts one 256-d k-pair in one MM; the first
            # opens the psum accumulation group.
            nc.tensor.matmul(
                qg_ps,
                (wsb if wsb is not None else bwT8_sb)[:, :, gc * 128:(gc + 1) * 128],
                qzT8[:, c0 // 16:(c0 + MEGA) // 16, :, :].rearrange(
                    "p blk two c -> p two blk c"
                ),
                start=start,
                stop=False,
                perf_mode=DR,
            )

        def mm1(t):
            # qgT[gc] = sum_k bwT[:,k,gc].T @ qzT[:,k,cols(t)] -> relu (ACT)
            qzT8, qzT8b, qzT = pairs[t // 2]
            c0 = (t % 2) * MEGA
            kb0 = KF8 if t < NDR23 else 0
            qgr = qgr_pool.tile([128, KG, MEGA], BF16, name="qgr")
            for gc in range(KG):
                qg_ps = qg_psum.tile([128, MEGA], F32, name="qg_ps")
                dr_mm1(qg_ps, qzT8, gc, c0)
                if t < NDR23:
                    dr_mm1(qg_ps, qzT8b, gc, c0, wsb=bwT8b_sb, start=False)
                for kb in range(kb0, KD - KF8):
                    nc.tensor.matmul(
                        qg_ps,
                        bwT_sb[:, kb, gc * 128:(gc + 1) * 128],
                        qzT[:, kb, c0:c0 + MEGA],
                        start=False,
                        stop=(kb == KD - KF8 - 1),
                    )
                nc.scalar.activation(qgr[:, gc, :], qg_ps, AF.Relu)
            return qgr

        def mm1_pair0():
            # Startup-only mm1 for megas 0 and 1: k-OUTER accumulation with
            # all 8 gc psum groups open at once (2 qg-pool banks + 6 msg-pool
            # banks incl. the warmup slot — the msg pool is idle until mm2(0)
            # ~12us later). Each k-quarter of pair0 is consumed the moment it
            # lands, so the PE streams real work through the whole DMA-bound
            # startup window with zero gc-outer data stalls.
            qzT8, qzT8b, qzT = pairs[0]
            banks = [
                [
                    qg_psum.tile([128, MEGA], F32, name="qg_ps"),
                    qg_psum.tile([128, MEGA], F32, name="qg_ps"),
                    msg_psum.tile([128, MEGA], F32, name="m_ps"),
                    msg_psum.tile([128, MEGA], F32, name="m_ps"),
                ],
                [
                    msg_psum.tile([128, MEGA], F32, name="m_ps"),
                    msg_psum.tile([128, MEGA], F32, name="m_ps"),
                    msg_psum.tile([128, MEGA], F32, name="m_ps"),
                    msg_psum.tile([128, MEGA], F32, name="m_ps"),
                ],
            ]
            qgrs = [
                qgr_pool.tile([128, KG, MEGA], BF16, name="qgr"),
                qgr_pool.tile([128, KG, MEGA], BF16, name="qgr"),
            ]
            for mega in range(2):
                for gc in range(KG):
                    dr_mm1(banks[mega][gc], qzT8, gc, mega * MEGA)
            for mega in range(2):
                for gc in range(KG):
                    dr_mm1(
                        banks[mega][gc], qzT8b, gc, mega * MEGA,
                        wsb=bwT8b_sb, start=False,
                    )
            for kb in range(KF8, KD - KF8):
                for mega in range(2):
                    for gc in range(KG):
                        nc.tensor.matmul(
                            banks[mega][gc],
                            bwT_sb[:, kb, gc * 128:(gc + 1) * 128],
                            qzT[:, kb, mega * MEGA:(mega + 1) * MEGA],
                            start=False,
                            stop=(kb == KD - KF8 - 1),
                        )
            # Mega-0 relus split across ACT and DVE (max(x,0) — identical
            # rounding) so mm2(0)'s stationary is ready ~2 relu-times after
            # the last k7 matmul instead of 4; mega-1's relus run on ACT
            # while mm2(0) computes.
            nc.scalar.activation(qgrs[0][:, 0, :], banks[0][0], AF.Relu)
            nc.vector.tensor_scalar_max(qgrs[0][:, 1, :], banks[0][1], 0.0)
            nc.scalar.activation(qgrs[0][:, 2, :], banks[0][2], AF.Relu)
            nc.vector.tensor_scalar_max(qgrs[0][:, 3, :], banks[0][3], 0.0)
            for gc in range(KG):
                nc.scalar.activation(qgrs[1][:, gc, :], banks[1][gc], AF.Relu)
            return qgrs

        def mm2_block(t, qgr):
            msg_sb = out_pool.tile([128, NSUB, D], BF16, name="msg_sb")

            def mmgroup(s, h):
                m_ps = msg_psum.tile([128, 512], F32, name="m_ps")
                for gc in range(KG):
                    nc.tensor.matmul(
                        m_ps,
                        qgr[:, gc, s * 128:(s + 1) * 128],
                        bw_sb[:, gc, h * 512:(h + 1) * 512],
                        start=(gc == 0),
                        stop=(gc == KG - 1),
                    )
                return m_ps

            def drain(s, h, m_ps, sc_sb):
                # all drains on DVE: ACT only runs the relus, so a drain is
                # never queued behind the next mega's relus on ACT's strict
                # FIFO (that ordering stalled mm2 psum-slot reuse by ~3us)
                dst = msg_sb[:, s, h * 512:(h + 1) * 512]
                nc.vector.tensor_scalar_mul(dst, m_ps, sc_sb[:, s:s + 1])

            # rowsum over g, den-direct: DVE sums the 4 qgr chunks into
            # acc [128(g_low), p] (bf16, error ~1e-3 of den — negligible),
            # then 4 tiny N=1 matmuls acc_chunk.T @ ones produce den for
            # each 128-row sub ALREADY in per-partition column layout.
            # This replaces 4 N=512 rowsum MMs + 4 PE transposes + a DVE
            # copy (~1.2us of PE per mega) with ~0.35us of PE.
            ADD = mybir.AluOpType.add
            s1 = small_pool.tile([128, MEGA], BF16, name="acc_s1")
            nc.vector.scalar_tensor_tensor(s1, qgr[:, 0, :], 0.0, qgr[:, 1, :], ADD, ADD)
            s2 = small_pool.tile([128, MEGA], BF16, name="acc_s2")
            nc.vector.scalar_tensor_tensor(s2, qgr[:, 2, :], 0.0, qgr[:, 3, :], ADD, ADD)
            acc = small_pool.tile([128, MEGA], BF16, name="acc")
            nc.vector.scalar_tensor_tensor(acc, s1, 0.0, s2, ADD, ADD)

            pending = [(0, 0, mmgroup(0, 0))]
            pending.append((0, 1, mmgroup(0, 1)))
            pending.append((1, 0, mmgroup(1, 0)))

            # sc_ps lives in the msg pool: its slot's previous occupant was
            # drained a full mega ago. (In the qg pool it reused a slot whose
            # last reader is mm1(t+1)'s relu — a ~0.4us/mega PE stall.) The
            # tiny sc matmuls sit after three mm2 groups so mega 0 — whose
            # DVE rowsum can only start at relu time — has its acc ready.
            sc_ps = msg_psum.tile([128, 512], F32, name="sc_ps", tag="m_ps")
            for ss in range(NSUB):
                nc.tensor.matmul(
                    sc_ps[:, ss:ss + 1],
                    acc[:, ss * 128:(ss + 1) * 128],
                    ones_g,
                )

            sc_sb = small_pool.tile([128, NSUB], F32, name="sc_sb")
            nc.vector.tensor_scalar_max(sc_sb, sc_ps[:, 0:NSUB], EPS)
            nc.vector.reciprocal(sc_sb, sc_sb)

            pending.append((1, 1, mmgroup(1, 1)))
            for (ps_, hs_, mp_) in pending:
                drain(ps_, hs_, mp_, sc_sb)

            last = t == NMEGA - 1
            if last:
                for s in (0, 1):
                    nc.sync.dma_start(
                        out=msg_d[t * MEGA + s * 128:t * MEGA + (s + 1) * 128, :],
                        in_=msg_sb[:, s, :],
                    )
            for s in (2, 3):
                if last and s == 3:
                    # Final sub: store in 512-col halves so the h0 half
                    # (128KB) is already in flight while h1 computes, and
                    # put the very last drain on ACT (no later relus exist
                    # to queue behind) so it starts the moment the last MM
                    # retires. Trims ~1us off the last-MM -> last-byte tail.
                    m0 = mmgroup(s, 0)
                    drain(s, 0, m0, sc_sb)
                    nc.sync.dma_start(
                        out=msg_d[t * MEGA + s * 128:t * MEGA + (s + 1) * 128, 0:512],
                        in_=msg_sb[:, s, 0:512],
                    )
                    m1 = mmgroup(s, 1)
                    nc.scalar.mul(msg_sb[:, s, 512:1024], m1, sc_sb[:, s:s + 1])
                    nc.sync.dma_start(
                        out=msg_d[t * MEGA + s * 128:t * MEGA + (s + 1) * 128, 512:1024],
                        in_=msg_sb[:, s, 512:1024],
                    )
                    continue
                for h in (0, 1):
                    drain(s, h, mmgroup(s, h), sc_sb)
                if last:
                    # per-sub stores at the end: the final store is only
                    # 256KB, shrinking the post-compute tail
                    nc.sync.dma_start(
                        out=msg_d[t * MEGA + s * 128:t * MEGA + (s + 1) * 128, :],
                        in_=msg_sb[:, s, :],
                    )
            if not last:
                # one store per mega: fewer ring-issue slots and completion
                # semaphores (the teardown epilogue scales with DMA count)
                nc.sync.dma_start(
                    out=msg_d[t * MEGA:(t + 1) * MEGA, :].rearrange(
                        "(s p) d -> p s d", p=128
                    ),
                    in_=msg_sb,
                )

        ensure_load(0)
        # Sync-ring order pair0, pair1, bw, pair2: mm1(2) is the first PE
        # work after the pair-0 k-outer stream (the pipeline runs two megas
        # ahead), so pair1 must not queue behind the 1MB bw load; bw itself
        # is only needed by mm2(0), a full mega of PE work later.
        ensure_load(1)
        nc.sync.dma_start(
            out=bw_sb, in_=bw_d[:].rearrange("(gc p) d -> p gc d", p=128)
        )
        ensure_load(2)
        qgr_queue = list(mm1_pair0())
        for t in range(NMEGA):
            nxt = t + 2
            if nxt < NMEGA:
                if nxt % 2 == 0:
                    ensure_load(nxt // 2 + 1)
                qgr_queue.append(mm1(nxt))
            mm2_block(t, qgr_queue.pop(0))

    nc.compile()
    return nc


_NC_CACHE = None


def _get_nc():
    global _NC_CACHE
    if _NC_CACHE is None:
        _NC_CACHE = build_kernel()
    return _NC_CACHE


def kernel(qz: np.ndarray, binary_weight: np.ndarray) -> np.ndarray:
    import ml_dtypes

    bf16 = ml_dtypes.bfloat16
    qz = np.asarray(qz, dtype=np.float32)
    bw32 = np.asarray(binary_weight, dtype=np.float32)
    assert qz.shape == (B, C, P, D), qz.shape
    assert bw32.shape == (B, G, D), bw32.shape
    bw = bw32.astype(bf16)

    fp8 = ml_dtypes.float8_e4m3fn if hasattr(ml_dtypes, "float8_e4m3fn") else ml_dtypes.float8_e4m3

    nc = _get_nc()
    in_maps = []
    for i in range(N_CORES):
        # qzT[p, j, k, c] = qz[j*1024+c, k*128+p]: contiguous per
        # (partition, pair) so pair DMAs are long-run descriptors.
        # k-chunks 0..1 ship as fp8e4 (DoubleRow operand, quantized straight
        # from fp32), chunks 2..7 as bf16.
        qzt = qz[i].reshape(R, D).reshape(NPAIR, 2 * MEGA, KD, 128).transpose(3, 0, 2, 1)
        qzT8 = np.ascontiguousarray(
            qzt[:, :, :KF8, :].reshape(128, NPAIR, KF8, 2 * MEGA // 16, 16)
            .transpose(0, 1, 3, 2, 4).astype(fp8)
        )
        qzT8b = np.ascontiguousarray(
            qzt[:, :NDR23 // 2, KF8:2 * KF8, :]
            .reshape(128, NDR23 // 2, KF8, 2 * MEGA // 16, 16)
            .transpose(0, 1, 3, 2, 4).astype(fp8)
        )
        qzT = np.ascontiguousarray(qzt[:, :, KF8:, :].astype(bf16))
        # bwT[p, k, g] = bw[g, k*128+p]
        bwt = bw32[i].reshape(G, KD, 128).transpose(2, 1, 0)
        bwT8 = np.ascontiguousarray(bwt[:, :KF8].astype(fp8))
        bwT8b = np.ascontiguousarray(bwt[:, KF8:2 * KF8].astype(fp8))
        bwT = np.ascontiguousarray(bwt[:, KF8:].astype(bf16))
        in_maps.append(
            {"qzT": qzT, "qzT8": qzT8, "qzT8b": qzT8b, "bw": bw[i],
             "bwT": bwT, "bwT8": bwT8, "bwT8b": bwT8b}
        )
    res = run_bass_kernel_spmd(nc, in_maps, core_ids=list(range(N_CORES)))
    out = np.stack(
        [
            res.results[i]["msg"].astype(np.float32).reshape(C, P, D)
            for i in range(N_CORES)
        ],
        axis=0,
    )
    return out
